# revision 10
# baseline (speedup 1.0000x reference)
"""Trainium2 Bass kernel for a small dense transformer block.

Module (hardcoded shapes): B=4, T=2048, D=64, H=8, FF=256.
  q/k/v: per-head full-width linears (H, D, D) + bias
  scores = q @ k.T (unscaled), causal, softmax
  out = attn @ v, concat heads -> proj (H*D -> D) + bias
  h1 = LN(x + attn_out); y = LN(h1 + relu(h1@W1+b1)@W2+b2)

Sharding: one head per core (8 heads / 8 cores). Each core computes its
head's attention and the partial projection attn_h @ (x @ Wv_h @ Wp_h);
per-batch ReduceScatters (bf16) sum partials over cores and shard each
batch's tokens 8 ways for the LN/FFN epilogue; the host reassembles.

Math folding (host-side, O(weights) only):
  q'_t = Wq.T x_t + bq, k'_s = Wk.T x_s + bk
  (k'_s)·(q'_t) = [k_s;1]·[q'_t; bk·q'_t]  -> biases folded into 65-dim
  augmented weights, contraction K=65 with a ones-row appended to x.T
  (x.T is transposed host-side and shipped bf16).
  softmax rows sum to 1 => v-bias and proj bias become the constant
  C = sum_h bv_h @ Wp_h + bp, added once in the epilogue.
  V'' gets a ones column so PV matmul also produces the softmax
  denominator (unnormalized accumulate, divide at the end).

Perf structure: all attention matmuls in bf16 (f32 PSUM accumulate);
t-blocks processed in pairs so each key-chunk's kT / v2 weight load is
shared by two matmuls; scores computed full-width so exp runs as
[128, 2, 512] paired instructions (amortizes ACT's fixed per-op cost);
causal masking via precomputed combined masks on DVE; PSUM->SBUF
copies on DVE (gpsimd cannot access PSUM); epilogue split in two
halves, the first interleaved with batch-3 attention.
"""

import numpy as np
import ml_dtypes

B, T, D, H, FF = 4, 2048, 64, 8, 256
NTOK = B * T          # 8192
SHARD = NTOK // 8     # 1024 (4 batches x 256 tokens per core)
QSH = T // 8          # 256: per-batch token shard per core
EPS = 1e-5
F32 = np.float32
BF16 = ml_dtypes.bfloat16

TB = 512              # t-block (query) width
NTB = T // TB         # 4 t-blocks per batch elem

_CACHE = {}


def _build_nc():
    import concourse.bass as bass
    import concourse.tile as tile
    from concourse import bacc, mybir

    f32 = mybir.dt.float32
    bf = mybir.dt.bfloat16
    Act = mybir.ActivationFunctionType
    Alu = mybir.AluOpType

    nc = bacc.Bacc("TRN2", target_bir_lowering=False, debug=False, num_devices=8)

    # ---- I/O ----
    xT_d = nc.dram_tensor("xT", [D + 1, NTOK], bf, kind="ExternalInput")
    xs_d = nc.dram_tensor("xs", [SHARD, D], f32, kind="ExternalInput")
    wqk_d = nc.dram_tensor("wqk", [D + 1, D + 1], bf, kind="ExternalInput")
    wkk_d = nc.dram_tensor("wkk", [D + 1, D + 1], bf, kind="ExternalInput")
    wvv_d = nc.dram_tensor("wvv", [D + 1, D + 2], bf, kind="ExternalInput")
    w1a_d = nc.dram_tensor("w1a", [D + 1, FF], bf, kind="ExternalInput")
    w2_d = nc.dram_tensor("w2", [FF, D], bf, kind="ExternalInput")
    # masks: mk[:, u, :] zeroes cols < 128u and applies the causal
    # triangle at cols [128u, 128u+128); passes cols >= 128(u+1).
    mk_d = nc.dram_tensor("mk", [128, 4, TB], bf, kind="ExternalInput")
    identb_d = nc.dram_tensor("identb", [128, 128], bf, kind="ExternalInput")
    ones_d = nc.dram_tensor("ones", [1, SHARD], bf, kind="ExternalInput")
    # broadcast constants, pre-replicated to 128 partitions on host
    cbc_d = nc.dram_tensor("cbc", [128, D], f32, kind="ExternalInput")
    b2bc_d = nc.dram_tensor("b2bc", [128, D], f32, kind="ExternalInput")
    g1bc_d = nc.dram_tensor("g1bc", [128, D], f32, kind="ExternalInput")
    be1bc_d = nc.dram_tensor("be1bc", [128, D], f32, kind="ExternalInput")
    g2bc_d = nc.dram_tensor("g2bc", [128, D], f32, kind="ExternalInput")
    be2bc_d = nc.dram_tensor("be2bc", [128, D], f32, kind="ExternalInput")
    out_d = nc.dram_tensor("out", [SHARD, D], f32, kind="ExternalOutput")

    with tile.TileContext(nc) as tc:
        with (
            tc.tile_pool(name="singles", bufs=1) as singles,
            tc.tile_pool(name="work", bufs=4) as work,
            tc.tile_pool(name="octt", bufs=2) as octt,
            tc.tile_pool(name="ep", bufs=2) as ep,
            tc.tile_pool(name="ps_s", bufs=3, space="PSUM") as ps_s,
            tc.tile_pool(name="ps_o", bufs=2, space="PSUM") as ps_o,
            tc.tile_pool(name="dram", bufs=1, space="DRAM") as dram,
        ):
            # ---- persistent SBUF ----
            xT = singles.tile([D + 1, NTOK], bf)      # x.T with ones row
            qT = singles.tile([D + 1, NTOK], bf)      # [q'; kappa]
            kT = singles.tile([D + 1, NTOK], bf)      # [k'; 1]
            v2 = singles.tile([128, NTOK // 128, D + 2], bf)
            mk = singles.tile([128, 4, TB], bf)       # causal masks
            identb = singles.tile([128, 128], bf)
            wqk = singles.tile([D + 1, D + 1], bf)
            wkk = singles.tile([D + 1, D + 1], bf)
            wvv = singles.tile([D + 1, D + 2], bf)
            w1a = singles.tile([D + 1, FF], bf)
            w2 = singles.tile([128, 2, D], bf)
            cbc = singles.tile([128, D], f32)
            b2bc = singles.tile([128, D], f32)
            g1bc = singles.tile([128, D], f32)
            be1bc = singles.tile([128, D], f32)
            g2bc = singles.tile([128, D], f32)
            be2bc = singles.tile([128, D], f32)
            epst = singles.tile([128, 1], f32)
            h1_all = singles.tile([128, SHARD // 128, D], f32)
            h1T = singles.tile([D + 1, SHARD], bf)
            f1rT = singles.tile([128, 2, SHARD], bf)
            xs_t = singles.tile([128, SHARD // 128, D], f32)
            rtb = singles.tile([128, SHARD // 128, D], bf)

            rs_in = [dram.tile([T, D], bf, tag=f"rsi{b}", name=f"rsi{b}")
                     for b in range(B)]
            rs_out = [dram.tile([QSH, D], bf, tag=f"rso{b}", name=f"rso{b}")
                      for b in range(B)]

            nc.sync.dma_start(mk[:], mk_d[:])
            nc.sync.dma_start(identb[:], identb_d[:])
            nc.sync.dma_start(wqk[:], wqk_d[:])
            nc.sync.dma_start(wkk[:], wkk_d[:])
            nc.sync.dma_start(wvv[:], wvv_d[:])
            nc.sync.dma_start(w1a[:], w1a_d[:])
            nc.sync.dma_start(w2[:], w2_d.rearrange("(c p) d -> p c d", p=128))
            nc.sync.dma_start(cbc[:], cbc_d[:])
            nc.sync.dma_start(b2bc[:], b2bc_d[:])
            nc.sync.dma_start(g1bc[:], g1bc_d[:])
            nc.sync.dma_start(be1bc[:], be1bc_d[:])
            nc.sync.dma_start(g2bc[:], g2bc_d[:])
            nc.sync.dma_start(be2bc[:], be2bc_d[:])
            nc.vector.memset(epst[:], EPS)
            nc.sync.dma_start(h1T[D : D + 1, :], ones_d[:, :])
            nc.sync.dma_start(
                xs_t[:], xs_d.rearrange("(q p) d -> p q d", p=128))
            # x.T arrives in per-batch slices so phase B can start early
            for b in range(B):
                nc.sync.dma_start(
                    xT[:, b * T : (b + 1) * T], xT_d[:, b * T : (b + 1) * T])

            def _phase_b(b):
                """qT/kT/v2 for batch b's tokens."""
                for dst, w in ((qT, wqk), (kT, wkk)):
                    for i in range(4 * b, 4 * b + 4):
                        pq = ps_o.tile([D + 1, TB], f32, tag="acc", name="pq")
                        nc.tensor.matmul(
                            pq[:], lhsT=w[:],
                            rhs=xT[:, TB * i : TB * (i + 1)],
                            start=True, stop=True,
                        )
                        nc.vector.tensor_copy(dst[:, TB * i : TB * (i + 1)], pq[:])
                for g in range(4 * b, 4 * b + 4):
                    pv = ps_s.tile([128, 4, D + 2], f32, tag="sT", name="pv")
                    for u in range(4):
                        i = 4 * g + u
                        nc.tensor.matmul(
                            pv[:, u, :], lhsT=xT[:, 128 * i : 128 * (i + 1)],
                            rhs=wvv[:],
                            start=True, stop=True,
                        )
                    nc.vector.tensor_copy(v2[:, 4 * g : 4 * (g + 1), :], pv[:])

            def _attn_batch(b):
                """Attention for batch b: t-block pairs (0,1), (2,3)."""
                base = b * T
                for jA in (0, 2):
                    jB = jA + 1
                    lastA, lastB = 4 * jA + 3, 4 * jB + 3
                    outs = {
                        jA: ps_o.tile([D + 1, TB], f32, tag="acc", name="outA"),
                        jB: ps_o.tile([D + 1, TB], f32, tag="acc", name="outB"),
                    }

                    def _pv(c, j, ex, idx):
                        last = lastA if j == jA else lastB
                        nc.tensor.matmul(
                            outs[j][:],
                            lhsT=v2[:, (base // 128) + c, : D + 1],
                            rhs=ex[:, idx, :],
                            start=(c == 0), stop=(c == last),
                        )

                    # units of two score-chunks sharing one exp instruction:
                    # (c, jA)+(c, jB) while both active, then the B-only
                    # tail pairs (c, jB)+(c+1, jB)
                    units = [((c, jA), (c, jB)) for c in range(lastA + 1)]
                    tail = list(range(lastA + 1, lastB + 1))
                    units += [
                        ((tail[i], jB), (tail[i + 1], jB))
                        for i in range(0, len(tail), 2)
                    ]

                    # score matmuls run LOOKAHEAD units ahead of the
                    # exp/PV consumers so the PE queue always holds ~3us
                    # of ready work (keeps the clock ramped)
                    LOOKAHEAD = 3
                    sTs = {}

                    def _scores(i):
                        sT = ps_s.tile([128, 2, TB], f32, tag="sT", name="sT")
                        for idx, (c, j) in enumerate(units[i]):
                            nc.tensor.matmul(
                                sT[:, idx, :],
                                lhsT=kT[:, base + 128 * c : base + 128 * (c + 1)],
                                rhs=qT[:, base + j * TB : base + (j + 1) * TB],
                                start=True, stop=True,
                            )
                        sTs[i] = sT

                    for i in range(min(LOOKAHEAD, len(units))):
                        _scores(i)
                    for i, unit in enumerate(units):
                        ex = work.tile([128, 2, TB], bf, tag="exp", name="ex")
                        nc.scalar.activation(ex[:], sTs.pop(i)[:], Act.Exp)
                        for idx, (c, j) in enumerate(unit):
                            u = c - 4 * j
                            if 0 <= u <= 3:
                                nc.vector.tensor_mul(
                                    ex[:, idx, :], ex[:, idx, :], mk[:, u, :])
                        for idx, (c, j) in enumerate(unit):
                            _pv(c, j, ex, idx)
                        if i + LOOKAHEAD < len(units):
                            _scores(i + LOOKAHEAD)

                    # drain: normalize + transpose to [t, d], ship to rs_in
                    for j in (jA, jB):
                        t0 = j * TB
                        oc = octt.tile([D + 1, TB], bf, tag="oc", name="oc")
                        nc.vector.tensor_copy(oc[:], outs[j][:])
                        tp = ps_s.tile([128, 4, D + 2], bf, tag="sT", name="tp")
                        part = work.tile([128, 4, D], bf, tag="part", name="part")
                        for u in range(4):
                            nc.tensor.transpose(
                                tp[:, u, : D + 1],
                                oc[:, 128 * u : 128 * (u + 1)],
                                identb[: D + 1, : D + 1],
                            )
                            rec = work.tile([128, 1], f32, tag="rec", name="rec")
                            nc.vector.reciprocal(rec[:], tp[:, u, D : D + 1])
                            nc.vector.tensor_scalar_mul(
                                part[:, u, :], tp[:, u, :D], rec[:])
                        nc.sync.dma_start(
                            rs_in[b][t0 : t0 + TB, :].rearrange(
                                "(u p) d -> p u d", p=128),
                            part[:],
                        )

            # ---- epilogue ----
            def _ln(z, dst, g, bb, nq):
                """dst = LN(z) * g + bb over the last dim; [128, nq, D]."""
                shape = [128, nq, D]
                mt = ep.tile([128, nq, 1], f32, tag="mt", name="mt")
                nc.vector.tensor_reduce(
                    mt[:], z, mybir.AxisListType.X, Alu.add)
                nc.vector.tensor_scalar_mul(mt[:], mt[:], 1.0 / D)
                nc.vector.tensor_tensor(
                    dst, z, mt.to_broadcast(shape), Alu.subtract)
                sq = ep.tile([128, nq, D], f32, tag="sq", name="sq")
                nc.vector.tensor_mul(sq[:], dst, dst)
                vt = ep.tile([128, nq, 1], f32, tag="vt", name="vt")
                nc.vector.tensor_reduce(
                    vt[:], sq[:], mybir.AxisListType.X, Alu.add)
                sd = ep.tile([128, nq, 1], f32, tag="sd", name="sd")
                nc.scalar.activation(
                    sd[:, :, 0], vt[:, :, 0], Act.Sqrt, bias=epst[:],
                    scale=1.0 / D)
                rc = ep.tile([128, nq, 1], f32, tag="rc", name="rc")
                nc.vector.reciprocal(rc[:], sd[:])
                nc.vector.tensor_tensor(
                    dst, dst, rc.to_broadcast(shape), Alu.mult)
                nc.vector.tensor_tensor(
                    dst, dst, g[:, None, :].to_broadcast(shape), Alu.mult)
                nc.vector.tensor_tensor(
                    dst, dst, bb[:, None, :].to_broadcast(shape), Alu.add)

            def _epilogue_quarter(q):
                """LN/FFN for shard tokens [256*q, 256*q+256) (batch q's
                token shard, available once batch q's RS lands)."""
                nq = 2
                t0 = QSH * q
                sl = (slice(None), slice(nq * q, nq * q + nq), slice(None))
                shape = [128, nq, D]
                nc.gpsimd.dma_start(
                    rtb[:, nq * q : nq * q + nq, :],
                    rs_out[q][:].rearrange("(c p) d -> p c d", p=128))
                zt = ep.tile([128, nq, D], f32, tag="zt", name="zt")
                nc.vector.tensor_tensor(zt[:], xs_t[sl], rtb[sl], Alu.add)
                nc.vector.tensor_tensor(
                    zt[:], zt[:], cbc[:, None, :].to_broadcast(shape), Alu.add)
                _ln(zt[:], h1_all[sl], g1bc, be1bc, nq)
                # bf16 copy + transpose to [d, t] layout for the FFN
                h1b = ep.tile([128, nq, D], bf, tag="h1b", name="h1b")
                nc.gpsimd.tensor_copy(h1b[:], h1_all[sl])
                tpe = ps_s.tile([D, QSH], bf, tag="sT", name="tpe")
                for u in range(nq):
                    nc.tensor.transpose(
                        tpe[:, 128 * u : 128 * (u + 1)], h1b[:, u, :],
                        identb[:])
                nc.vector.tensor_copy(h1T[:D, t0 : t0 + QSH], tpe[:])
                # FFN up + relu
                for fc in range(2):
                    pf = ps_o.tile([128, QSH], f32, tag="acc", name="pf")
                    nc.tensor.matmul(
                        pf[:],
                        lhsT=w1a[:, 128 * fc : 128 * (fc + 1)],
                        rhs=h1T[:, t0 : t0 + QSH],
                        start=True, stop=True,
                    )
                    nc.scalar.activation(
                        f1rT[:, fc, t0 : t0 + QSH], pf[:], Act.Relu)
                # FFN down + residual + LN2
                y_all = ep.tile([128, nq, D], f32, tag="yt", name="y_all")
                for u in range(nq):
                    ch = nq * q + u
                    p2 = ps_s.tile([128, D], f32, tag="sT", name="p2")
                    nc.tensor.matmul(
                        p2[:], lhsT=f1rT[:, 0, 128 * ch : 128 * (ch + 1)],
                        rhs=w2[:, 0, :],
                        start=True, stop=False,
                    )
                    nc.tensor.matmul(
                        p2[:], lhsT=f1rT[:, 1, 128 * ch : 128 * (ch + 1)],
                        rhs=w2[:, 1, :],
                        start=False, stop=True,
                    )
                    nc.vector.tensor_copy(y_all[:, u, :], p2[:])
                nc.vector.tensor_tensor(
                    y_all[:], y_all[:],
                    b2bc[:, None, :].to_broadcast(shape), Alu.add)
                nc.vector.tensor_tensor(y_all[:], y_all[:], h1_all[sl], Alu.add)
                o_all = ep.tile([128, nq, D], f32, tag="ot", name="o_all")
                _ln(y_all[:], o_all[:], g2bc, be2bc, nq)
                nc.gpsimd.dma_start(
                    out_d[t0 : t0 + QSH, :].rearrange(
                        "(c p) d -> p c d", p=128),
                    o_all[:])

            # ---- schedule ----
            # per-batch ReduceScatter right after each batch's attention;
            # epilogue quarters all after batch 3 (quarters 0-2 execute
            # while batch 3's RS is in flight; only quarter 3 waits on it)
            _phase_b(0)
            for b in range(B):
                if b + 1 < B:
                    _phase_b(b + 1)
                _attn_batch(b)
                nc.gpsimd.collective_compute(
                    "ReduceScatter",
                    Alu.add,
                    replica_groups=[list(range(8))],
                    ins=[rs_in[b][:]],
                    outs=[rs_out[b][:]],
                )
            for q in range(B):
                _epilogue_quarter(q)

    nc.compile()
    return nc


def _prep_inputs(inputs, Wq, bq, Wk, bk, Wv, bv, Wp, bp, W1, b1, W2, b2,
                 g1, be1, g2, be2):
    """Host-side input prep: augmented per-head weights + per-core maps."""
    x = np.ascontiguousarray(np.asarray(inputs, dtype=F32).reshape(NTOK, D))
    Wq, bq = np.asarray(Wq, F32), np.asarray(bq, F32)
    Wk, bk = np.asarray(Wk, F32), np.asarray(bk, F32)
    Wv, bv = np.asarray(Wv, F32), np.asarray(bv, F32)
    Wp, bp = np.asarray(Wp, F32), np.asarray(bp, F32)

    bc = lambda v: np.ascontiguousarray(
        np.broadcast_to(np.asarray(v, F32).reshape(1, D), (128, D))
    )
    # x.T with ones row, bf16
    xT = np.concatenate([x.T, np.ones((1, NTOK), F32)], axis=0)
    xT = np.ascontiguousarray(xT.astype(BF16))
    # masks: mk[p, u, t] = 0 for t < 128u; causal triangle (key p visible
    # to query t, i.e. t >= p) within [128u, 128u+128); 1 beyond.
    tri = np.triu(np.ones((128, 128), F32))
    mk = np.zeros((128, 4, TB), F32)
    for u in range(4):
        mk[:, u, 128 * u : 128 * (u + 1)] = tri
        mk[:, u, 128 * (u + 1) :] = 1.0
    mk = np.ascontiguousarray(mk.astype(BF16))
    identb = np.ascontiguousarray(np.eye(128, dtype=F32).astype(BF16))
    ones = np.ones((1, SHARD), BF16)

    C = sum(
        bv[h].astype(np.float64) @ Wp[D * h : D * (h + 1)].astype(np.float64)
        for h in range(H)
    ) + bp.astype(np.float64)

    common = dict(
        xT=xT, mk=mk, identb=identb, ones=ones,
        w1a=np.ascontiguousarray(np.concatenate(
            [np.asarray(W1, F32), np.asarray(b1, F32).reshape(1, FF)],
            axis=0).astype(BF16)),
        w2=np.ascontiguousarray(np.asarray(W2, F32).astype(BF16)),
        cbc=bc(C.astype(F32)), b2bc=bc(b2),
        g1bc=bc(g1), be1bc=bc(be1), g2bc=bc(g2), be2bc=bc(be2),
    )

    e64 = np.zeros((D + 1, 1), F32)
    e64[D, 0] = 1.0
    in_maps = []
    for h in range(H):
        wq_aug = np.concatenate([Wq[h], bq[h].reshape(1, D)], axis=0)  # [65, 64]
        kappa = (wq_aug.astype(np.float64) @ bk[h].astype(np.float64)).astype(F32)
        wqk = np.concatenate([wq_aug, kappa.reshape(D + 1, 1)], axis=1)
        wk_aug = np.concatenate([Wk[h], bk[h].reshape(1, D)], axis=0)
        wkk = np.concatenate([wk_aug, e64], axis=1)
        wvp = (Wv[h].astype(np.float64)
               @ Wp[D * h : D * (h + 1)].astype(np.float64)).astype(F32)
        wvv = np.concatenate(
            [np.concatenate([wvp, np.zeros((1, D), F32)], axis=0), e64,
             np.zeros((D + 1, 1), F32)], axis=1)
        # per-core token shard: for each batch b, rows [b*T + QSH*h, +QSH)
        xs_h = np.concatenate(
            [x[b * T + QSH * h : b * T + QSH * (h + 1)] for b in range(B)])
        in_maps.append(dict(
            common,
            xs=np.ascontiguousarray(xs_h),
            wqk=np.ascontiguousarray(wqk.astype(BF16)),
            wkk=np.ascontiguousarray(wkk.astype(BF16)),
            wvv=np.ascontiguousarray(wvv.astype(BF16)),
        ))
    return in_maps


def _get_nc():
    if "nc" not in _CACHE:
        _CACHE["nc"] = _build_nc()
    return _CACHE["nc"]


def kernel(**inputs) -> np.ndarray:
    from concourse.bass_utils import run_bass_kernel_spmd

    in_maps = _prep_inputs(**inputs)
    nc = _get_nc()
    res = run_bass_kernel_spmd(nc, in_maps, list(range(8)))
    out = np.empty((NTOK, D), F32)
    for c in range(8):
        shard = res.results[c]["out"]
        for b in range(B):
            out[b * T + QSH * c : b * T + QSH * (c + 1)] = (
                shard[QSH * b : QSH * (b + 1)])
    return out.reshape(B, T, D)
